# revision 8
# baseline (speedup 1.0000x reference)
"""DecisionBoundary loss kernel for TRN2, 8 NeuronCores, pure data-parallel.

Math (per row, C=1000 classes):
    prob   = softmax(x)
    out    = 1 + max_{c != y} prob_c - prob_y

Rewritten without explicit softmax materialization (x ~ N(0,1) so exp(x)
is safe in fp32 without max-subtraction):
    e     = exp(x)                 (ACT engine, accum -> s = sum(e))
    r     = (iota != y) * e        (DVE scalar_tensor_tensor; accum -> s - e_y)
    w     = max(r)                 (DVE tensor_scalar with max-accum; exact
                                    because e > 0 strictly and r_y = 0)
    out   = 1 + (w - e_y) / s  =  1 + (w - s + sA) / s,   sA = s - e_y

Sharding: batch axis split 8 ways (32768 rows/core), 256 tiles of
[128 rows x 1000 classes] per core. No cross-core communication.
"""

import numpy as np

import concourse.bacc as bacc
import concourse.bass as bass
import concourse.tile as tile
from concourse import mybir
from concourse.bass_utils import run_bass_kernel_spmd

BATCH = 262144
C = 1000
NCORES = 8
ROWS = BATCH // NCORES  # 32768 rows per core
P = 128                 # SBUF partitions (rows per tile)

_cache: dict = {}


def build_nc(rows: int = ROWS, ncols: int = C, x_bufs: int = 18, repeat: int = 1,
             variant: str = "full", ctype: str = "float16"):
    """Build the per-core Bass program (SPMD: same program on all cores).

    repeat > 1 wraps the whole body in an on-device loop (benchmarking only:
    one NEFF, `repeat` full passes over the same data).
    variant: "full" | "dma" | "dma_act" | "dma_dve" (bench-only ablations).
    ctype: dtype of the exp/masked tensors feeding the DVE ops. float16
      enables the DVE 2-byte perf modes; integers 0..999 and per-tile sums
      stay exact enough (accumulators are always fp32).
    """
    ntiles = rows // P
    nc = bacc.Bacc("TRN2")

    x_d = nc.dram_tensor("x", [rows, ncols], mybir.dt.float32, kind="ExternalInput")
    y_d = nc.dram_tensor("y", [P, ntiles], mybir.dt.float32, kind="ExternalInput")
    out_d = nc.dram_tensor("out", [P, ntiles], mybir.dt.float32, kind="ExternalOutput")

    fp32 = mybir.dt.float32
    cdt = getattr(mybir.dt, ctype)
    Alu = mybir.AluOpType

    with tile.TileContext(nc) as tc:
        with (
            tc.tile_pool(name="xs", bufs=x_bufs) as xs_pool,
            tc.tile_pool(name="es", bufs=4) as es_pool,
            tc.tile_pool(name="rs", bufs=4) as rs_pool,
            tc.tile_pool(name="const", bufs=1) as const_pool,
            tc.tile_pool(name="acc", bufs=1) as acc_pool,
        ):
            # Constants: iota along the class dim (same for every partition),
            # and the per-tile labels y (one column per tile).
            iota_i = const_pool.tile([P, ncols], mybir.dt.int32)
            nc.gpsimd.iota(iota_i[:, :], [[1, ncols]], channel_multiplier=0)
            iota_f = const_pool.tile([P, ncols], cdt)
            nc.vector.tensor_copy(iota_f[:, :], iota_i[:, :])

            y_sb = const_pool.tile([P, ntiles], fp32)
            nc.sync.dma_start(out=y_sb[:, :], in_=y_d[:, :])

            # Per-tile scalar accumulators, one column per tile.
            s_acc = acc_pool.tile([P, ntiles], fp32)   # sum(exp(x))
            sA_acc = acc_pool.tile([P, ntiles], fp32)  # s - exp(x_y)
            w_acc = acc_pool.tile([P, ntiles], fp32)   # max_{c != y} exp(x_c)

            # Dummy broadcast targets for the unused full-size outputs.
            dummy1 = const_pool.tile([P, 8], cdt)
            dummy2 = const_pool.tile([P, 8], fp32)

            if variant != "full":
                nc.vector.memset(s_acc[:, :], 1.0)
                nc.vector.memset(sA_acc[:, :], 1.0)
                nc.vector.memset(w_acc[:, :], 1.0)

            def emit_body():
                for t in range(ntiles):
                    xt = xs_pool.tile([P, ncols], fp32)
                    nc.sync.dma_start(out=xt[:, :], in_=x_d[t * P:(t + 1) * P, :])

                    if variant in ("full", "dma_act"):
                        et = es_pool.tile([P, ncols], cdt)
                        nc.scalar.activation(
                            out=et[:, :],
                            in_=xt[:, :],
                            func=mybir.ActivationFunctionType.Exp,
                            accum_out=s_acc[:, t:t + 1],
                        )

                    if variant in ("full", "dma_dve"):
                        src = et if variant == "full" else xt
                        rt = rs_pool.tile([P, ncols], cdt)
                        nc.vector.scalar_tensor_tensor(
                            out=rt[:, :],
                            in0=iota_f[:, :],
                            scalar=y_sb[:, t:t + 1],
                            in1=src[:, :],
                            op0=Alu.not_equal,
                            op1=Alu.mult,
                            accum_out=sA_acc[:, t:t + 1],
                        )

                        nc.vector.tensor_scalar(
                            out=dummy1[:, 0:1].broadcast_to((P, ncols)),
                            in0=rt[:, :],
                            scalar1=0.0,
                            scalar2=None,
                            op0=Alu.add,
                            op1=Alu.max,
                            accum_out=w_acc[:, t:t + 1],
                        )

                # out = 1 + (w - s + sA) / s
                num = acc_pool.tile([P, ntiles], fp32)
                nc.vector.tensor_tensor(
                    num[:, :], w_acc[:, :], s_acc[:, :], Alu.subtract)
                nc.vector.tensor_tensor(num[:, :], num[:, :], sA_acc[:, :], Alu.add)
                rcp = acc_pool.tile([P, ntiles], fp32)
                nc.vector.reciprocal(rcp[:, :], s_acc[:, :])
                prod = acc_pool.tile([P, ntiles], fp32)
                nc.vector.tensor_tensor(prod[:, :], num[:, :], rcp[:, :], Alu.mult)
                outb = acc_pool.tile([P, ntiles], fp32)
                nc.vector.tensor_scalar(
                    out=outb[:, :], in0=prod[:, :], scalar1=1.0, scalar2=None,
                    op0=Alu.add,
                )
                nc.sync.dma_start(out=out_d[:, :], in_=outb[:, :])

            if repeat > 1:
                with tc.For_i(0, repeat, 1):
                    emit_body()
            else:
                emit_body()
            _ = dummy2  # reserved
    if not nc.is_finalized():
        nc.finalize()
    return nc


def build_nc_v3(rows: int = ROWS, ncols: int = C, x_bufs: int = 18,
                repeat: int = 1, variant: str = "full", gather_splits: int = 8):
    """v3: per tile only ACT (exp+accum) and DVE (InstMax top-8).

    x_y is fetched by indirect DMA (one 4-byte gather per row, batched into
    gather_splits instructions); e_y = exp(x_y) recomputed by ACT bit-exactly,
    so (e_y == m) identifies y == argmax and selects max vs second max.
    """
    ntiles = rows // P
    nc = bacc.Bacc("TRN2", num_swdge_queues=4)

    x_d = nc.dram_tensor("x", [rows, ncols], mybir.dt.float32, kind="ExternalInput")
    y_d = nc.dram_tensor("y", [P, ntiles], mybir.dt.int32, kind="ExternalInput")
    out_d = nc.dram_tensor("out", [P, ntiles], mybir.dt.float32, kind="ExternalOutput")
    x_flat = x_d.rearrange("r (c one) -> (r c) one", one=1)

    fp32 = mybir.dt.float32
    i32 = mybir.dt.int32
    Alu = mybir.AluOpType

    with tile.TileContext(nc) as tc:
        with (
            tc.tile_pool(name="xs", bufs=x_bufs) as xs_pool,
            tc.tile_pool(name="es", bufs=4) as es_pool,
            tc.tile_pool(name="const", bufs=1) as const_pool,
            tc.tile_pool(name="acc", bufs=1) as acc_pool,
        ):
            # y_d carries host-marshalled flat offsets: (t*128 + p)*ncols + y.
            off_sb = const_pool.tile([P, ntiles], i32)
            nc.sync.dma_start(out=off_sb[:, :], in_=y_d[:, :])

            s_acc = acc_pool.tile([P, ntiles], fp32)     # sum(exp(x)) per row
            top8 = acc_pool.tile([P, 8 * ntiles], fp32)  # top-8 of exp(x) per tile
            xg = acc_pool.tile([P, ntiles], fp32)        # gathered x_y per row

            if variant in ("dma", "dma_act", "dma_dve"):
                nc.vector.memset(s_acc[:, :], 1.0)
                nc.vector.memset(top8[:, :], 1.0)
                nc.vector.memset(xg[:, :], 0.0)

            def emit_body():
                if variant in ("full", "dma_dve"):
                    # HW indirect DMA consumes ONE index per partition
                    # descriptor (gathers out-free-size contiguous elements),
                    # so gather per tile: out/indices [128, 1].
                    for t in range(ntiles):
                        nc.gpsimd.indirect_dma_start(
                            out=xg[:, t:t + 1],
                            out_offset=None,
                            in_=x_flat,
                            in_offset=bass.IndirectOffsetOnAxis(
                                ap=off_sb[:, t:t + 1], axis=0),
                        )
                for t in range(ntiles):
                    xt = xs_pool.tile([P, ncols], fp32)
                    nc.sync.dma_start(out=xt[:, :], in_=x_d[t * P:(t + 1) * P, :])

                    if variant in ("full", "dma_act"):
                        et = es_pool.tile([P, ncols], fp32)
                        nc.scalar.activation(
                            out=et[:, :],
                            in_=xt[:, :],
                            func=mybir.ActivationFunctionType.Exp,
                            accum_out=s_acc[:, t:t + 1],
                        )
                    if variant in ("full", "dma_dve"):
                        # Top-8 of the raw logits: values are exact DRAM bits,
                        # so (x_y == max) below is an exact argmax test.
                        nc.vector.max(out=top8[:, 8 * t:8 * t + 8], in_=xt[:, :])
                        _ = et

                # Batched epilogue over [P, ntiles]:
                ey = acc_pool.tile([P, ntiles], fp32)
                nc.scalar.activation(ey[:, :], xg[:, :],
                                     func=mybir.ActivationFunctionType.Exp)
                t8v = top8[:, :].rearrange("p (t e) -> p t e", e=8)
                m1 = t8v[:, :, 0]   # max logit
                m2 = t8v[:, :, 1]   # second max logit
                eq = acc_pool.tile([P, ntiles], fp32)
                nc.vector.tensor_tensor(eq[:, :], xg[:, :], m1, Alu.is_equal)
                d12 = acc_pool.tile([P, ntiles], fp32)
                nc.vector.tensor_tensor(d12[:, :], m1, m2, Alu.subtract)
                eqd = acc_pool.tile([P, ntiles], fp32)
                nc.vector.tensor_tensor(eqd[:, :], eq[:, :], d12[:, :], Alu.mult)
                wl = acc_pool.tile([P, ntiles], fp32)   # max wrong-class logit
                nc.vector.tensor_tensor(wl[:, :], m1, eqd[:, :], Alu.subtract)
                we = acc_pool.tile([P, ntiles], fp32)   # exp of it
                nc.scalar.activation(we[:, :], wl[:, :],
                                     func=mybir.ActivationFunctionType.Exp)
                num = acc_pool.tile([P, ntiles], fp32)
                nc.vector.tensor_tensor(num[:, :], we[:, :], ey[:, :], Alu.subtract)
                rcp = acc_pool.tile([P, ntiles], fp32)
                nc.vector.reciprocal(rcp[:, :], s_acc[:, :])
                prod = acc_pool.tile([P, ntiles], fp32)
                nc.vector.tensor_tensor(prod[:, :], num[:, :], rcp[:, :], Alu.mult)
                outb = acc_pool.tile([P, ntiles], fp32)
                nc.vector.tensor_scalar(
                    out=outb[:, :], in0=prod[:, :], scalar1=1.0, scalar2=None,
                    op0=Alu.add,
                )
                nc.sync.dma_start(out=out_d[:, :], in_=outb[:, :])

            if repeat > 1:
                with tc.For_i(0, repeat, 1):
                    emit_body()
            else:
                emit_body()
    if not nc.is_finalized():
        nc.finalize()
    return nc


def build_nc_v5(rows: int = ROWS, ncols: int = C, x_bufs: int = 18,
                repeat: int = 1, variant: str = "full", gather_frac: float = 0.66):
    """v5 hybrid: x_y via pool indirect-DMA gather for the first
    gather_frac of tiles, via a DVE one-hot stt (exact on logits) for the
    rest; InstMax top-8 on raw logits for every tile; exp+accum on ACT.

    y input [P, 2T] int32: cols 0..T-1 flat gather offsets, T..2T-1 y values.
    """
    ntiles = rows // P
    tg = int(round(ntiles * gather_frac))
    nc = bacc.Bacc("TRN2", num_swdge_queues=4)

    x_d = nc.dram_tensor("x", [rows, ncols], mybir.dt.float32, kind="ExternalInput")
    y_d = nc.dram_tensor("y", [P, 2 * ntiles], mybir.dt.int32, kind="ExternalInput")
    out_d = nc.dram_tensor("out", [P, ntiles], mybir.dt.float32, kind="ExternalOutput")
    x_flat = x_d.rearrange("r (c one) -> (r c) one", one=1)

    fp32 = mybir.dt.float32
    i32 = mybir.dt.int32
    Alu = mybir.AluOpType

    with tile.TileContext(nc) as tc:
        with (
            tc.tile_pool(name="xs", bufs=x_bufs) as xs_pool,
            tc.tile_pool(name="es", bufs=3) as es_pool,
            tc.tile_pool(name="const", bufs=1) as const_pool,
            tc.tile_pool(name="acc", bufs=1) as acc_pool,
        ):
            y2_sb = const_pool.tile([P, 2 * ntiles], i32)
            nc.sync.dma_start(out=y2_sb[:, :], in_=y_d[:, :])
            off_sb = y2_sb[:, 0:ntiles]
            y_f = const_pool.tile([P, ntiles], fp32)
            nc.vector.tensor_copy(y_f[:, :], y2_sb[:, ntiles:2 * ntiles])

            iota_i = const_pool.tile([P, ncols], i32)
            nc.gpsimd.iota(iota_i[:, :], [[1, ncols]], channel_multiplier=0)
            iota_f = const_pool.tile([P, ncols], fp32)
            nc.vector.tensor_copy(iota_f[:, :], iota_i[:, :])

            s_acc = acc_pool.tile([P, ntiles], fp32)
            top8 = acc_pool.tile([P, 8 * ntiles], fp32)
            # Separate buffers for the two x_y paths: no cross-engine false
            # deps between pool gathers and DVE stt accums. Merged by add in
            # the epilogue (unwritten columns are memset to 0).
            xg_g = acc_pool.tile([P, ntiles], fp32)
            xg_s = acc_pool.tile([P, ntiles], fp32)
            dummy1 = const_pool.tile([P, 8], fp32)

            # Interleave: stt on every 3rd tile keeps DVE demand uniform
            # (~1.6us/tile) instead of spiking to 2.4us/tile in a tail.
            def is_stt(t):
                return (t % 3 == 2) if 0.0 < gather_frac < 1.0 else \
                    (t >= tg)

            def emit_body():
                nc.vector.memset(xg_g[:, :], 0.0)
                nc.vector.memset(xg_s[:, :], 0.0)
                for t in range(ntiles):
                    if not is_stt(t):
                        nc.gpsimd.indirect_dma_start(
                            out=xg_g[:, t:t + 1],
                            out_offset=None,
                            in_=x_flat,
                            in_offset=bass.IndirectOffsetOnAxis(
                                ap=off_sb[:, t:t + 1], axis=0),
                        )
                for t in range(ntiles):
                    xt = xs_pool.tile([P, ncols], fp32)
                    nc.sync.dma_start(out=xt[:, :], in_=x_d[t * P:(t + 1) * P, :])

                    et = es_pool.tile([P, ncols], fp32)
                    nc.scalar.activation(
                        out=et[:, :],
                        in_=xt[:, :],
                        func=mybir.ActivationFunctionType.Exp,
                        accum_out=s_acc[:, t:t + 1],
                    )
                    nc.vector.max(out=top8[:, 8 * t:8 * t + 8], in_=xt[:, :])
                    if is_stt(t):
                        # x_y = sum(onehot(y) * x): exact (zeros elsewhere).
                        nc.vector.scalar_tensor_tensor(
                            out=dummy1[:, 0:1].broadcast_to((P, ncols)),
                            in0=iota_f[:, :],
                            scalar=y_f[:, t:t + 1],
                            in1=xt[:, :],
                            op0=Alu.is_equal,
                            op1=Alu.mult,
                            accum_out=xg_s[:, t:t + 1],
                        )

                # Batched epilogue over [P, ntiles]:
                xg = acc_pool.tile([P, ntiles], fp32)
                nc.vector.tensor_tensor(xg[:, :], xg_g[:, :], xg_s[:, :], Alu.add)
                ey = acc_pool.tile([P, ntiles], fp32)
                nc.scalar.activation(ey[:, :], xg[:, :],
                                     func=mybir.ActivationFunctionType.Exp)
                t8v = top8[:, :].rearrange("p (t e) -> p t e", e=8)
                m1 = t8v[:, :, 0]
                m2 = t8v[:, :, 1]
                eq = acc_pool.tile([P, ntiles], fp32)
                nc.vector.tensor_tensor(eq[:, :], xg[:, :], m1, Alu.is_equal)
                d12 = acc_pool.tile([P, ntiles], fp32)
                nc.vector.tensor_tensor(d12[:, :], m1, m2, Alu.subtract)
                eqd = acc_pool.tile([P, ntiles], fp32)
                nc.vector.tensor_tensor(eqd[:, :], eq[:, :], d12[:, :], Alu.mult)
                wl = acc_pool.tile([P, ntiles], fp32)
                nc.vector.tensor_tensor(wl[:, :], m1, eqd[:, :], Alu.subtract)
                we = acc_pool.tile([P, ntiles], fp32)
                nc.scalar.activation(we[:, :], wl[:, :],
                                     func=mybir.ActivationFunctionType.Exp)
                num = acc_pool.tile([P, ntiles], fp32)
                nc.vector.tensor_tensor(num[:, :], we[:, :], ey[:, :], Alu.subtract)
                rcp = acc_pool.tile([P, ntiles], fp32)
                nc.vector.reciprocal(rcp[:, :], s_acc[:, :])
                prod = acc_pool.tile([P, ntiles], fp32)
                nc.vector.tensor_tensor(prod[:, :], num[:, :], rcp[:, :], Alu.mult)
                outb = acc_pool.tile([P, ntiles], fp32)
                nc.vector.tensor_scalar(
                    out=outb[:, :], in0=prod[:, :], scalar1=1.0, scalar2=None,
                    op0=Alu.add,
                )
                nc.sync.dma_start(out=out_d[:, :], in_=outb[:, :])

            if repeat > 1:
                with tc.For_i(0, repeat, 1):
                    emit_body()
            else:
                emit_body()
            _ = variant
    if not nc.is_finalized():
        nc.finalize()
    return nc


def build_nc_v6(rows: int = ROWS, ncols: int = C, x_bufs: int = 20,
                repeat: int = 1, variant: str = "full", stt_mod: int = 3,
                e_bufs: int = 4):
    """v6: fp16 exp tiles so every DVE op runs in a 2-byte perf mode.

    Per tile: ACT exp fp32->fp16 (accum fp32 -> s), DVE InstMax top-8 on the
    fp16 e tile.  e_y: pool indirect-DMA gather of x_y (exp'd in the
    epilogue, bit-identically via the same ACT path) for most tiles, fp16
    one-hot stt accum (= e_y directly, exact) on every stt_mod-th tile.
    Epilogue identifies y == argmax by e_y == m1 (exact in fp16) and picks
    m2 in that case.  y input layout identical to v5 ([P, 2T] int32).
    """
    ntiles = rows // P
    nc = bacc.Bacc("TRN2", num_swdge_queues=4)

    x_d = nc.dram_tensor("x", [rows, ncols], mybir.dt.float32, kind="ExternalInput")
    y_d = nc.dram_tensor("y", [P, 2 * ntiles], mybir.dt.int32, kind="ExternalInput")
    out_d = nc.dram_tensor("out", [P, ntiles], mybir.dt.float32, kind="ExternalOutput")
    x_flat = x_d.rearrange("r (c one) -> (r c) one", one=1)

    fp32 = mybir.dt.float32
    fp16 = mybir.dt.float16
    i32 = mybir.dt.int32
    Alu = mybir.AluOpType

    def is_stt(t):
        return stt_mod > 0 and t % stt_mod == stt_mod - 1

    with tile.TileContext(nc) as tc:
        with (
            tc.tile_pool(name="xs", bufs=x_bufs) as xs_pool,
            tc.tile_pool(name="es", bufs=e_bufs) as es_pool,
            tc.tile_pool(name="const", bufs=1) as const_pool,
            tc.tile_pool(name="acc", bufs=1) as acc_pool,
        ):
            y2_sb = const_pool.tile([P, 2 * ntiles], i32)
            nc.sync.dma_start(out=y2_sb[:, :], in_=y_d[:, :])
            off_sb = y2_sb[:, 0:ntiles]
            y_f = const_pool.tile([P, ntiles], fp32)
            nc.vector.tensor_copy(y_f[:, :], y2_sb[:, ntiles:2 * ntiles])

            iota_i = const_pool.tile([P, ncols], i32)
            nc.gpsimd.iota(iota_i[:, :], [[1, ncols]], channel_multiplier=0)
            iota16 = const_pool.tile([P, ncols], fp16)
            nc.vector.tensor_copy(iota16[:, :], iota_i[:, :])

            s_acc = acc_pool.tile([P, ntiles], fp32)
            top8 = acc_pool.tile([P, 8 * ntiles], fp16)
            xg_g = acc_pool.tile([P, ntiles], fp32)   # gathered x_y (-1e4 at stt cols)
            ey_s = acc_pool.tile([P, ntiles], fp32)   # stt-path e_y (0 at gather cols)
            dummy16 = const_pool.tile([P, 8], fp16)

            def emit_body():
                nc.vector.memset(xg_g[:, :], -10000.0)
                nc.vector.memset(ey_s[:, :], 0.0)
                if variant in ("full", "dma_dve"):
                    for t in range(ntiles):
                        if not is_stt(t):
                            nc.gpsimd.indirect_dma_start(
                                out=xg_g[:, t:t + 1],
                                out_offset=None,
                                in_=x_flat,
                                in_offset=bass.IndirectOffsetOnAxis(
                                    ap=off_sb[:, t:t + 1], axis=0),
                            )
                for t in range(ntiles):
                    xt = xs_pool.tile([P, ncols], fp32)
                    nc.sync.dma_start(out=xt[:, :], in_=x_d[t * P:(t + 1) * P, :])

                    if variant in ("full", "dma_act"):
                        et = es_pool.tile([P, ncols], fp16)
                        nc.scalar.activation(
                            out=et[:, :],
                            in_=xt[:, :],
                            func=mybir.ActivationFunctionType.Exp,
                            accum_out=s_acc[:, t:t + 1],
                        )
                    if variant == "full":
                        nc.vector.max(out=top8[:, 8 * t:8 * t + 8], in_=et[:, :])
                        if is_stt(t):
                            nc.vector.scalar_tensor_tensor(
                                out=dummy16[:, 0:1].broadcast_to((P, ncols)),
                                in0=iota16[:, :],
                                scalar=y_f[:, t:t + 1],
                                in1=et[:, :],
                                op0=Alu.is_equal,
                                op1=Alu.mult,
                                accum_out=ey_s[:, t:t + 1],
                            )

                # Batched epilogue over [P, ntiles]:
                ey_g16 = acc_pool.tile([P, ntiles], fp16)
                nc.scalar.activation(ey_g16[:, :], xg_g[:, :],
                                     func=mybir.ActivationFunctionType.Exp)
                ey_g = acc_pool.tile([P, ntiles], fp32)
                nc.vector.tensor_copy(ey_g[:, :], ey_g16[:, :])
                ey = acc_pool.tile([P, ntiles], fp32)
                nc.vector.tensor_tensor(ey[:, :], ey_g[:, :], ey_s[:, :], Alu.add)

                t8v = top8[:, :].rearrange("p (t e) -> p t e", e=8)
                m1f = acc_pool.tile([P, ntiles], fp32)
                nc.vector.tensor_copy(m1f[:, :], t8v[:, :, 0])
                m2f = acc_pool.tile([P, ntiles], fp32)
                nc.vector.tensor_copy(m2f[:, :], t8v[:, :, 1])

                eq = acc_pool.tile([P, ntiles], fp32)
                nc.vector.tensor_tensor(eq[:, :], ey[:, :], m1f[:, :], Alu.is_equal)
                d12 = acc_pool.tile([P, ntiles], fp32)
                nc.vector.tensor_tensor(d12[:, :], m1f[:, :], m2f[:, :], Alu.subtract)
                eqd = acc_pool.tile([P, ntiles], fp32)
                nc.vector.tensor_tensor(eqd[:, :], eq[:, :], d12[:, :], Alu.mult)
                w = acc_pool.tile([P, ntiles], fp32)
                nc.vector.tensor_tensor(w[:, :], m1f[:, :], eqd[:, :], Alu.subtract)

                num = acc_pool.tile([P, ntiles], fp32)
                nc.vector.tensor_tensor(num[:, :], w[:, :], ey[:, :], Alu.subtract)
                rcp = acc_pool.tile([P, ntiles], fp32)
                nc.vector.reciprocal(rcp[:, :], s_acc[:, :])
                prod = acc_pool.tile([P, ntiles], fp32)
                nc.vector.tensor_tensor(prod[:, :], num[:, :], rcp[:, :], Alu.mult)
                outb = acc_pool.tile([P, ntiles], fp32)
                nc.vector.tensor_scalar(
                    out=outb[:, :], in0=prod[:, :], scalar1=1.0, scalar2=None,
                    op0=Alu.add,
                )
                nc.sync.dma_start(out=out_d[:, :], in_=outb[:, :])

            if variant in ("dma", "dma_act", "dma_dve"):
                nc.vector.memset(s_acc[:, :], 1.0)
                nc.vector.memset(top8[:, :], 1.0)
            if repeat > 1:
                with tc.For_i(0, repeat, 1):
                    emit_body()
            else:
                emit_body()
    if not nc.is_finalized():
        nc.finalize()
    return nc


def build_nc_v7(rows: int = ROWS, ncols: int = C, x_bufs: int = 6,
                repeat: int = 1, variant: str = "full", stt_mod: int = 0,
                e_bufs: int = 3, rpp: int = 4, dma2q: bool = False):
    """v7: v6 compute with big DMA transfers (rpp rows per partition).

    One dma_start moves rpp*512KB: partition p holds rows rpp*P*bt + rpp*p
    .. +rpp-1 contiguously (8/16KB per partition line).  Compute runs per
    1000-col slice exactly as v6; out col c = rpp*bt + h covers rows
    rpp*P*bt + rpp*p + h.  Host marshalling must use y_dtype="v7-<rpp>".
    """
    ntiles = rows // P
    nbig = ntiles // rpp
    nc = bacc.Bacc("TRN2", num_swdge_queues=4)

    x_d = nc.dram_tensor("x", [rows, ncols], mybir.dt.float32, kind="ExternalInput")
    y_d = nc.dram_tensor("y", [P, 2 * ntiles], mybir.dt.int32, kind="ExternalInput")
    out_d = nc.dram_tensor("out", [P, ntiles], mybir.dt.float32, kind="ExternalOutput")
    x_flat = x_d.rearrange("r (c one) -> (r c) one", one=1)
    xb = x_d.rearrange("(n p k) c -> n p (k c)", p=P, k=rpp)

    fp32 = mybir.dt.float32
    fp16 = mybir.dt.float16
    i32 = mybir.dt.int32
    Alu = mybir.AluOpType

    def is_stt(t):
        return stt_mod > 0 and t % stt_mod == stt_mod - 1

    with tile.TileContext(nc) as tc:
        with (
            tc.tile_pool(name="xs", bufs=x_bufs) as xs_pool,
            tc.tile_pool(name="es", bufs=e_bufs) as es_pool,
            tc.tile_pool(name="const", bufs=1) as const_pool,
            tc.tile_pool(name="acc", bufs=1) as acc_pool,
        ):
            y2_sb = const_pool.tile([P, 2 * ntiles], i32)
            nc.sync.dma_start(out=y2_sb[:, :], in_=y_d[:, :])
            off_sb = y2_sb[:, 0:ntiles]
            y_f = const_pool.tile([P, ntiles], fp32)
            nc.vector.tensor_copy(y_f[:, :], y2_sb[:, ntiles:2 * ntiles])

            iota_i = const_pool.tile([P, ncols], i32)
            nc.gpsimd.iota(iota_i[:, :], [[1, ncols]], channel_multiplier=0)
            iota16 = const_pool.tile([P, ncols], fp16)
            nc.vector.tensor_copy(iota16[:, :], iota_i[:, :])

            s_acc = acc_pool.tile([P, ntiles], fp32)
            top8 = acc_pool.tile([P, 8 * ntiles], fp16)
            xg_g = acc_pool.tile([P, ntiles], fp32)
            ey_s = acc_pool.tile([P, ntiles], fp32)
            dummy16 = const_pool.tile([P, 8], fp16)

            def emit_body():
                nc.vector.memset(xg_g[:, :], -10000.0)
                nc.vector.memset(ey_s[:, :], 0.0)
                if variant in ("full", "dma_dve"):
                    for t in range(ntiles):
                        if not is_stt(t):
                            nc.gpsimd.indirect_dma_start(
                                out=xg_g[:, t:t + 1],
                                out_offset=None,
                                in_=x_flat,
                                in_offset=bass.IndirectOffsetOnAxis(
                                    ap=off_sb[:, t:t + 1], axis=0),
                            )
                for bt in range(nbig):
                    xt = xs_pool.tile([P, rpp * ncols], fp32)
                    eng = nc.scalar if (dma2q and bt % 2) else nc.sync
                    eng.dma_start(out=xt[:, :], in_=xb[bt])

                    if variant in ("full", "dma_act"):
                        et = es_pool.tile([P, rpp * ncols], fp16)
                        for h in range(rpp):
                            t = rpp * bt + h
                            sl = slice(h * ncols, (h + 1) * ncols)
                            nc.scalar.activation(
                                out=et[:, sl],
                                in_=xt[:, sl],
                                func=mybir.ActivationFunctionType.Exp,
                                accum_out=s_acc[:, t:t + 1],
                            )
                    if variant == "full":
                        for h in range(rpp):
                            t = rpp * bt + h
                            sl = slice(h * ncols, (h + 1) * ncols)
                            nc.vector.max(out=top8[:, 8 * t:8 * t + 8],
                                          in_=et[:, sl])
                            if is_stt(t):
                                nc.vector.scalar_tensor_tensor(
                                    out=dummy16[:, 0:1].broadcast_to((P, ncols)),
                                    in0=iota16[:, :],
                                    scalar=y_f[:, t:t + 1],
                                    in1=et[:, sl],
                                    op0=Alu.is_equal,
                                    op1=Alu.mult,
                                    accum_out=ey_s[:, t:t + 1],
                                )

                # Batched epilogue over [P, ntiles]:
                ey_g16 = acc_pool.tile([P, ntiles], fp16)
                nc.scalar.activation(ey_g16[:, :], xg_g[:, :],
                                     func=mybir.ActivationFunctionType.Exp)
                ey_g = acc_pool.tile([P, ntiles], fp32)
                nc.vector.tensor_copy(ey_g[:, :], ey_g16[:, :])
                ey = acc_pool.tile([P, ntiles], fp32)
                nc.vector.tensor_tensor(ey[:, :], ey_g[:, :], ey_s[:, :], Alu.add)

                t8v = top8[:, :].rearrange("p (t e) -> p t e", e=8)
                m1f = acc_pool.tile([P, ntiles], fp32)
                nc.vector.tensor_copy(m1f[:, :], t8v[:, :, 0])
                m2f = acc_pool.tile([P, ntiles], fp32)
                nc.vector.tensor_copy(m2f[:, :], t8v[:, :, 1])

                eq = acc_pool.tile([P, ntiles], fp32)
                nc.vector.tensor_tensor(eq[:, :], ey[:, :], m1f[:, :], Alu.is_equal)
                d12 = acc_pool.tile([P, ntiles], fp32)
                nc.vector.tensor_tensor(d12[:, :], m1f[:, :], m2f[:, :], Alu.subtract)
                eqd = acc_pool.tile([P, ntiles], fp32)
                nc.vector.tensor_tensor(eqd[:, :], eq[:, :], d12[:, :], Alu.mult)
                w = acc_pool.tile([P, ntiles], fp32)
                nc.vector.tensor_tensor(w[:, :], m1f[:, :], eqd[:, :], Alu.subtract)

                num = acc_pool.tile([P, ntiles], fp32)
                nc.vector.tensor_tensor(num[:, :], w[:, :], ey[:, :], Alu.subtract)
                rcp = acc_pool.tile([P, ntiles], fp32)
                nc.vector.reciprocal(rcp[:, :], s_acc[:, :])
                prod = acc_pool.tile([P, ntiles], fp32)
                nc.vector.tensor_tensor(prod[:, :], num[:, :], rcp[:, :], Alu.mult)
                outb = acc_pool.tile([P, ntiles], fp32)
                nc.vector.tensor_scalar(
                    out=outb[:, :], in0=prod[:, :], scalar1=1.0, scalar2=None,
                    op0=Alu.add,
                )
                nc.sync.dma_start(out=out_d[:, :], in_=outb[:, :])

            if variant in ("dma", "dma_act", "dma_dve"):
                nc.vector.memset(s_acc[:, :], 1.0)
                nc.vector.memset(top8[:, :], 1.0)
            if repeat > 1:
                with tc.For_i(0, repeat, 1):
                    emit_body()
            else:
                emit_body()
    if not nc.is_finalized():
        nc.finalize()
    return nc


def make_in_maps(state_output: np.ndarray, y: np.ndarray, y_dtype=np.int32,
                 rows: int = ROWS, ncols: int = C):
    """Shard the full inputs across cores (batch split + y marshalling).

    For the v3 kernel (y_dtype=int32) the y input carries flat element
    offsets local_row*ncols + y, the gather table for the indirect DMA.
    """
    x_full = np.ascontiguousarray(np.asarray(state_output, dtype=np.float32))
    y_full = np.asarray(y)
    ncores = y_full.shape[0] // rows
    in_maps = []
    for i in range(ncores):
        lo, hi = i * rows, (i + 1) * rows
        y_shard = y_full[lo:hi]
        if isinstance(y_dtype, str) and y_dtype.startswith("v7-"):
            rpp = int(y_dtype.split("-")[1])
            ntiles = rows // P
            r_mat = row_map_v7(rows, rpp)                   # [P, T] row index
            offs = (r_mat.astype(np.int64) * ncols
                    + y_shard.astype(np.int64)[r_mat]).astype(np.int32)
            vals = y_shard.astype(np.int32)[r_mat]
            y_t = np.ascontiguousarray(
                np.concatenate([offs, vals], axis=1))       # [P, 2T]
        elif y_dtype == "v5":
            offs = (np.arange(rows, dtype=np.int64) * ncols
                    + y_shard.astype(np.int64)).astype(np.int32)
            o_t = offs.reshape(rows // P, P).T              # [P, T]
            v_t = y_shard.astype(np.int32).reshape(rows // P, P).T
            y_t = np.ascontiguousarray(np.concatenate([o_t, v_t], axis=1))
        elif y_dtype == np.int32:
            vals = (np.arange(rows, dtype=np.int64) * ncols
                    + y_shard.astype(np.int64)).astype(np.int32)
            y_t = np.ascontiguousarray(vals.reshape(rows // P, P).T)  # [P, T]
        else:
            vals = y_shard.astype(y_dtype)
            y_t = np.ascontiguousarray(vals.reshape(rows // P, P).T)  # [P, T]
        in_maps.append({"x": x_full[lo:hi], "y": y_t})
    return in_maps


KERNEL_VERSION = "v7-4"   # "v5" / "v3" / "v6" fallbacks; "v7-<rpp>" big-DMA


def row_map_v7(rows: int, rpp: int) -> np.ndarray:
    """[P, T] matrix: local row index held at (partition p, out col c)."""
    ntiles = rows // P
    bt = np.arange(ntiles) // rpp
    h = np.arange(ntiles) % rpp
    return (rpp * P * bt)[None, :] + rpp * np.arange(P)[:, None] + h[None, :]


def y_dtype_for(version: str):
    if version.startswith("v7-"):
        return version
    return "v5" if version in ("v5", "v6") else np.int32


def build_for(version: str, **kw):
    if version.startswith("v7-"):
        return build_nc_v7(rpp=int(version.split("-")[1]), **kw)
    return globals()[f"build_nc_{version}"](**kw)


def unshard_out(o: np.ndarray, version: str) -> np.ndarray:
    """Per-core [P, T] device output -> flat [ROWS] local losses."""
    if version.startswith("v7-"):
        rpp = int(version.split("-")[1])
        ntiles = o.shape[1]
        o3 = o.reshape(P, ntiles // rpp, rpp)
        return o3.transpose(1, 0, 2).reshape(-1)
    return o.T.reshape(-1)


def kernel(state_output: np.ndarray, y: np.ndarray) -> np.ndarray:
    if "nc" not in _cache:
        _cache["nc"] = build_for(KERNEL_VERSION)
    nc = _cache["nc"]
    in_maps = make_in_maps(state_output, y, y_dtype=y_dtype_for(KERNEL_VERSION))
    res = run_bass_kernel_spmd(nc, in_maps, core_ids=list(range(NCORES)))
    outs = []
    for i in range(NCORES):
        o = np.asarray(res.results[i]["out"])  # [P, T]
        outs.append(unshard_out(o, KERNEL_VERSION))
    return np.concatenate(outs).astype(np.float32)



# revision 12
# speedup vs baseline: 1.9299x; 1.9299x over previous
"""DecisionBoundary loss kernel for TRN2, 8 NeuronCores, pure data-parallel.

Math (per row, C=1000 classes):
    prob   = softmax(x)
    out    = 1 + max_{c != y} prob_c - prob_y

Rewritten without explicit softmax materialization (x ~ N(0,1) so exp(x)
is safe in fp32 without max-subtraction):
    e     = exp(x)                 (ACT engine, accum -> s = sum(e))
    r     = (iota != y) * e        (DVE scalar_tensor_tensor; accum -> s - e_y)
    w     = max(r)                 (DVE tensor_scalar with max-accum; exact
                                    because e > 0 strictly and r_y = 0)
    out   = 1 + (w - e_y) / s  =  1 + (w - s + sA) / s,   sA = s - e_y

Sharding: batch axis split 8 ways (32768 rows/core), 256 tiles of
[128 rows x 1000 classes] per core. No cross-core communication.
"""

import numpy as np

import concourse.bacc as bacc
import concourse.bass as bass
import concourse.tile as tile
from concourse import mybir
from concourse.bass_utils import run_bass_kernel_spmd

BATCH = 262144
C = 1000
NCORES = 8
ROWS = BATCH // NCORES  # 32768 rows per core
P = 128                 # SBUF partitions (rows per tile)

_cache: dict = {}


def build_nc(rows: int = ROWS, ncols: int = C, x_bufs: int = 18, repeat: int = 1,
             variant: str = "full", ctype: str = "float16"):
    """Build the per-core Bass program (SPMD: same program on all cores).

    repeat > 1 wraps the whole body in an on-device loop (benchmarking only:
    one NEFF, `repeat` full passes over the same data).
    variant: "full" | "dma" | "dma_act" | "dma_dve" (bench-only ablations).
    ctype: dtype of the exp/masked tensors feeding the DVE ops. float16
      enables the DVE 2-byte perf modes; integers 0..999 and per-tile sums
      stay exact enough (accumulators are always fp32).
    """
    ntiles = rows // P
    nc = bacc.Bacc("TRN2")

    x_d = nc.dram_tensor("x", [rows, ncols], mybir.dt.float32, kind="ExternalInput")
    y_d = nc.dram_tensor("y", [P, ntiles], mybir.dt.float32, kind="ExternalInput")
    out_d = nc.dram_tensor("out", [P, ntiles], mybir.dt.float32, kind="ExternalOutput")

    fp32 = mybir.dt.float32
    cdt = getattr(mybir.dt, ctype)
    Alu = mybir.AluOpType

    with tile.TileContext(nc) as tc:
        with (
            tc.tile_pool(name="xs", bufs=x_bufs) as xs_pool,
            tc.tile_pool(name="es", bufs=4) as es_pool,
            tc.tile_pool(name="rs", bufs=4) as rs_pool,
            tc.tile_pool(name="const", bufs=1) as const_pool,
            tc.tile_pool(name="acc", bufs=1) as acc_pool,
        ):
            # Constants: iota along the class dim (same for every partition),
            # and the per-tile labels y (one column per tile).
            iota_i = const_pool.tile([P, ncols], mybir.dt.int32)
            nc.gpsimd.iota(iota_i[:, :], [[1, ncols]], channel_multiplier=0)
            iota_f = const_pool.tile([P, ncols], cdt)
            nc.vector.tensor_copy(iota_f[:, :], iota_i[:, :])

            y_sb = const_pool.tile([P, ntiles], fp32)
            nc.sync.dma_start(out=y_sb[:, :], in_=y_d[:, :])

            # Per-tile scalar accumulators, one column per tile.
            s_acc = acc_pool.tile([P, ntiles], fp32)   # sum(exp(x))
            sA_acc = acc_pool.tile([P, ntiles], fp32)  # s - exp(x_y)
            w_acc = acc_pool.tile([P, ntiles], fp32)   # max_{c != y} exp(x_c)

            # Dummy broadcast targets for the unused full-size outputs.
            dummy1 = const_pool.tile([P, 8], cdt)
            dummy2 = const_pool.tile([P, 8], fp32)

            if variant != "full":
                nc.vector.memset(s_acc[:, :], 1.0)
                nc.vector.memset(sA_acc[:, :], 1.0)
                nc.vector.memset(w_acc[:, :], 1.0)

            def emit_body():
                for t in range(ntiles):
                    xt = xs_pool.tile([P, ncols], fp32)
                    nc.sync.dma_start(out=xt[:, :], in_=x_d[t * P:(t + 1) * P, :])

                    if variant in ("full", "dma_act"):
                        et = es_pool.tile([P, ncols], cdt)
                        nc.scalar.activation(
                            out=et[:, :],
                            in_=xt[:, :],
                            func=mybir.ActivationFunctionType.Exp,
                            accum_out=s_acc[:, t:t + 1],
                        )

                    if variant in ("full", "dma_dve"):
                        src = et if variant == "full" else xt
                        rt = rs_pool.tile([P, ncols], cdt)
                        nc.vector.scalar_tensor_tensor(
                            out=rt[:, :],
                            in0=iota_f[:, :],
                            scalar=y_sb[:, t:t + 1],
                            in1=src[:, :],
                            op0=Alu.not_equal,
                            op1=Alu.mult,
                            accum_out=sA_acc[:, t:t + 1],
                        )

                        nc.vector.tensor_scalar(
                            out=dummy1[:, 0:1].broadcast_to((P, ncols)),
                            in0=rt[:, :],
                            scalar1=0.0,
                            scalar2=None,
                            op0=Alu.add,
                            op1=Alu.max,
                            accum_out=w_acc[:, t:t + 1],
                        )

                # out = 1 + (w - s + sA) / s
                num = acc_pool.tile([P, ntiles], fp32)
                nc.vector.tensor_tensor(
                    num[:, :], w_acc[:, :], s_acc[:, :], Alu.subtract)
                nc.vector.tensor_tensor(num[:, :], num[:, :], sA_acc[:, :], Alu.add)
                rcp = acc_pool.tile([P, ntiles], fp32)
                nc.vector.reciprocal(rcp[:, :], s_acc[:, :])
                prod = acc_pool.tile([P, ntiles], fp32)
                nc.vector.tensor_tensor(prod[:, :], num[:, :], rcp[:, :], Alu.mult)
                outb = acc_pool.tile([P, ntiles], fp32)
                nc.vector.tensor_scalar(
                    out=outb[:, :], in0=prod[:, :], scalar1=1.0, scalar2=None,
                    op0=Alu.add,
                )
                nc.sync.dma_start(out=out_d[:, :], in_=outb[:, :])

            if repeat > 1:
                with tc.For_i(0, repeat, 1):
                    emit_body()
            else:
                emit_body()
            _ = dummy2  # reserved
    if not nc.is_finalized():
        nc.finalize()
    return nc


def build_nc_v3(rows: int = ROWS, ncols: int = C, x_bufs: int = 18,
                repeat: int = 1, variant: str = "full", gather_splits: int = 8):
    """v3: per tile only ACT (exp+accum) and DVE (InstMax top-8).

    x_y is fetched by indirect DMA (one 4-byte gather per row, batched into
    gather_splits instructions); e_y = exp(x_y) recomputed by ACT bit-exactly,
    so (e_y == m) identifies y == argmax and selects max vs second max.
    """
    ntiles = rows // P
    nc = bacc.Bacc("TRN2", num_swdge_queues=4)

    x_d = nc.dram_tensor("x", [rows, ncols], mybir.dt.float32, kind="ExternalInput")
    y_d = nc.dram_tensor("y", [P, ntiles], mybir.dt.int32, kind="ExternalInput")
    out_d = nc.dram_tensor("out", [P, ntiles], mybir.dt.float32, kind="ExternalOutput")
    x_flat = x_d.rearrange("r (c one) -> (r c) one", one=1)

    fp32 = mybir.dt.float32
    i32 = mybir.dt.int32
    Alu = mybir.AluOpType

    with tile.TileContext(nc) as tc:
        with (
            tc.tile_pool(name="xs", bufs=x_bufs) as xs_pool,
            tc.tile_pool(name="es", bufs=4) as es_pool,
            tc.tile_pool(name="const", bufs=1) as const_pool,
            tc.tile_pool(name="acc", bufs=1) as acc_pool,
        ):
            # y_d carries host-marshalled flat offsets: (t*128 + p)*ncols + y.
            off_sb = const_pool.tile([P, ntiles], i32)
            nc.sync.dma_start(out=off_sb[:, :], in_=y_d[:, :])

            s_acc = acc_pool.tile([P, ntiles], fp32)     # sum(exp(x)) per row
            top8 = acc_pool.tile([P, 8 * ntiles], fp32)  # top-8 of exp(x) per tile
            xg = acc_pool.tile([P, ntiles], fp32)        # gathered x_y per row

            if variant in ("dma", "dma_act", "dma_dve"):
                nc.vector.memset(s_acc[:, :], 1.0)
                nc.vector.memset(top8[:, :], 1.0)
                nc.vector.memset(xg[:, :], 0.0)

            def emit_body():
                if variant in ("full", "dma_dve"):
                    # HW indirect DMA consumes ONE index per partition
                    # descriptor (gathers out-free-size contiguous elements),
                    # so gather per tile: out/indices [128, 1].
                    for t in range(ntiles):
                        nc.gpsimd.indirect_dma_start(
                            out=xg[:, t:t + 1],
                            out_offset=None,
                            in_=x_flat,
                            in_offset=bass.IndirectOffsetOnAxis(
                                ap=off_sb[:, t:t + 1], axis=0),
                        )
                for t in range(ntiles):
                    xt = xs_pool.tile([P, ncols], fp32)
                    nc.sync.dma_start(out=xt[:, :], in_=x_d[t * P:(t + 1) * P, :])

                    if variant in ("full", "dma_act"):
                        et = es_pool.tile([P, ncols], fp32)
                        nc.scalar.activation(
                            out=et[:, :],
                            in_=xt[:, :],
                            func=mybir.ActivationFunctionType.Exp,
                            accum_out=s_acc[:, t:t + 1],
                        )
                    if variant in ("full", "dma_dve"):
                        # Top-8 of the raw logits: values are exact DRAM bits,
                        # so (x_y == max) below is an exact argmax test.
                        nc.vector.max(out=top8[:, 8 * t:8 * t + 8], in_=xt[:, :])
                        _ = et

                # Batched epilogue over [P, ntiles]:
                ey = acc_pool.tile([P, ntiles], fp32)
                nc.scalar.activation(ey[:, :], xg[:, :],
                                     func=mybir.ActivationFunctionType.Exp)
                t8v = top8[:, :].rearrange("p (t e) -> p t e", e=8)
                m1 = t8v[:, :, 0]   # max logit
                m2 = t8v[:, :, 1]   # second max logit
                eq = acc_pool.tile([P, ntiles], fp32)
                nc.vector.tensor_tensor(eq[:, :], xg[:, :], m1, Alu.is_equal)
                d12 = acc_pool.tile([P, ntiles], fp32)
                nc.vector.tensor_tensor(d12[:, :], m1, m2, Alu.subtract)
                eqd = acc_pool.tile([P, ntiles], fp32)
                nc.vector.tensor_tensor(eqd[:, :], eq[:, :], d12[:, :], Alu.mult)
                wl = acc_pool.tile([P, ntiles], fp32)   # max wrong-class logit
                nc.vector.tensor_tensor(wl[:, :], m1, eqd[:, :], Alu.subtract)
                we = acc_pool.tile([P, ntiles], fp32)   # exp of it
                nc.scalar.activation(we[:, :], wl[:, :],
                                     func=mybir.ActivationFunctionType.Exp)
                num = acc_pool.tile([P, ntiles], fp32)
                nc.vector.tensor_tensor(num[:, :], we[:, :], ey[:, :], Alu.subtract)
                rcp = acc_pool.tile([P, ntiles], fp32)
                nc.vector.reciprocal(rcp[:, :], s_acc[:, :])
                prod = acc_pool.tile([P, ntiles], fp32)
                nc.vector.tensor_tensor(prod[:, :], num[:, :], rcp[:, :], Alu.mult)
                outb = acc_pool.tile([P, ntiles], fp32)
                nc.vector.tensor_scalar(
                    out=outb[:, :], in0=prod[:, :], scalar1=1.0, scalar2=None,
                    op0=Alu.add,
                )
                nc.sync.dma_start(out=out_d[:, :], in_=outb[:, :])

            if repeat > 1:
                with tc.For_i(0, repeat, 1):
                    emit_body()
            else:
                emit_body()
    if not nc.is_finalized():
        nc.finalize()
    return nc


def build_nc_v5(rows: int = ROWS, ncols: int = C, x_bufs: int = 18,
                repeat: int = 1, variant: str = "full", gather_frac: float = 0.66):
    """v5 hybrid: x_y via pool indirect-DMA gather for the first
    gather_frac of tiles, via a DVE one-hot stt (exact on logits) for the
    rest; InstMax top-8 on raw logits for every tile; exp+accum on ACT.

    y input [P, 2T] int32: cols 0..T-1 flat gather offsets, T..2T-1 y values.
    """
    ntiles = rows // P
    tg = int(round(ntiles * gather_frac))
    nc = bacc.Bacc("TRN2", num_swdge_queues=4)

    x_d = nc.dram_tensor("x", [rows, ncols], mybir.dt.float32, kind="ExternalInput")
    y_d = nc.dram_tensor("y", [P, 2 * ntiles], mybir.dt.int32, kind="ExternalInput")
    out_d = nc.dram_tensor("out", [P, ntiles], mybir.dt.float32, kind="ExternalOutput")
    x_flat = x_d.rearrange("r (c one) -> (r c) one", one=1)

    fp32 = mybir.dt.float32
    i32 = mybir.dt.int32
    Alu = mybir.AluOpType

    with tile.TileContext(nc) as tc:
        with (
            tc.tile_pool(name="xs", bufs=x_bufs) as xs_pool,
            tc.tile_pool(name="es", bufs=3) as es_pool,
            tc.tile_pool(name="const", bufs=1) as const_pool,
            tc.tile_pool(name="acc", bufs=1) as acc_pool,
        ):
            y2_sb = const_pool.tile([P, 2 * ntiles], i32)
            nc.sync.dma_start(out=y2_sb[:, :], in_=y_d[:, :])
            off_sb = y2_sb[:, 0:ntiles]
            y_f = const_pool.tile([P, ntiles], fp32)
            nc.vector.tensor_copy(y_f[:, :], y2_sb[:, ntiles:2 * ntiles])

            iota_i = const_pool.tile([P, ncols], i32)
            nc.gpsimd.iota(iota_i[:, :], [[1, ncols]], channel_multiplier=0)
            iota_f = const_pool.tile([P, ncols], fp32)
            nc.vector.tensor_copy(iota_f[:, :], iota_i[:, :])

            s_acc = acc_pool.tile([P, ntiles], fp32)
            top8 = acc_pool.tile([P, 8 * ntiles], fp32)
            # Separate buffers for the two x_y paths: no cross-engine false
            # deps between pool gathers and DVE stt accums. Merged by add in
            # the epilogue (unwritten columns are memset to 0).
            xg_g = acc_pool.tile([P, ntiles], fp32)
            xg_s = acc_pool.tile([P, ntiles], fp32)
            dummy1 = const_pool.tile([P, 8], fp32)

            # Interleave: stt on every 3rd tile keeps DVE demand uniform
            # (~1.6us/tile) instead of spiking to 2.4us/tile in a tail.
            def is_stt(t):
                return (t % 3 == 2) if 0.0 < gather_frac < 1.0 else \
                    (t >= tg)

            def emit_body():
                nc.vector.memset(xg_g[:, :], 0.0)
                nc.vector.memset(xg_s[:, :], 0.0)
                for t in range(ntiles):
                    if not is_stt(t):
                        nc.gpsimd.indirect_dma_start(
                            out=xg_g[:, t:t + 1],
                            out_offset=None,
                            in_=x_flat,
                            in_offset=bass.IndirectOffsetOnAxis(
                                ap=off_sb[:, t:t + 1], axis=0),
                        )
                for t in range(ntiles):
                    xt = xs_pool.tile([P, ncols], fp32)
                    nc.sync.dma_start(out=xt[:, :], in_=x_d[t * P:(t + 1) * P, :])

                    et = es_pool.tile([P, ncols], fp32)
                    nc.scalar.activation(
                        out=et[:, :],
                        in_=xt[:, :],
                        func=mybir.ActivationFunctionType.Exp,
                        accum_out=s_acc[:, t:t + 1],
                    )
                    nc.vector.max(out=top8[:, 8 * t:8 * t + 8], in_=xt[:, :])
                    if is_stt(t):
                        # x_y = sum(onehot(y) * x): exact (zeros elsewhere).
                        nc.vector.scalar_tensor_tensor(
                            out=dummy1[:, 0:1].broadcast_to((P, ncols)),
                            in0=iota_f[:, :],
                            scalar=y_f[:, t:t + 1],
                            in1=xt[:, :],
                            op0=Alu.is_equal,
                            op1=Alu.mult,
                            accum_out=xg_s[:, t:t + 1],
                        )

                # Batched epilogue over [P, ntiles]:
                xg = acc_pool.tile([P, ntiles], fp32)
                nc.vector.tensor_tensor(xg[:, :], xg_g[:, :], xg_s[:, :], Alu.add)
                ey = acc_pool.tile([P, ntiles], fp32)
                nc.scalar.activation(ey[:, :], xg[:, :],
                                     func=mybir.ActivationFunctionType.Exp)
                t8v = top8[:, :].rearrange("p (t e) -> p t e", e=8)
                m1 = t8v[:, :, 0]
                m2 = t8v[:, :, 1]
                eq = acc_pool.tile([P, ntiles], fp32)
                nc.vector.tensor_tensor(eq[:, :], xg[:, :], m1, Alu.is_equal)
                d12 = acc_pool.tile([P, ntiles], fp32)
                nc.vector.tensor_tensor(d12[:, :], m1, m2, Alu.subtract)
                eqd = acc_pool.tile([P, ntiles], fp32)
                nc.vector.tensor_tensor(eqd[:, :], eq[:, :], d12[:, :], Alu.mult)
                wl = acc_pool.tile([P, ntiles], fp32)
                nc.vector.tensor_tensor(wl[:, :], m1, eqd[:, :], Alu.subtract)
                we = acc_pool.tile([P, ntiles], fp32)
                nc.scalar.activation(we[:, :], wl[:, :],
                                     func=mybir.ActivationFunctionType.Exp)
                num = acc_pool.tile([P, ntiles], fp32)
                nc.vector.tensor_tensor(num[:, :], we[:, :], ey[:, :], Alu.subtract)
                rcp = acc_pool.tile([P, ntiles], fp32)
                nc.vector.reciprocal(rcp[:, :], s_acc[:, :])
                prod = acc_pool.tile([P, ntiles], fp32)
                nc.vector.tensor_tensor(prod[:, :], num[:, :], rcp[:, :], Alu.mult)
                outb = acc_pool.tile([P, ntiles], fp32)
                nc.vector.tensor_scalar(
                    out=outb[:, :], in0=prod[:, :], scalar1=1.0, scalar2=None,
                    op0=Alu.add,
                )
                nc.sync.dma_start(out=out_d[:, :], in_=outb[:, :])

            if repeat > 1:
                with tc.For_i(0, repeat, 1):
                    emit_body()
            else:
                emit_body()
            _ = variant
    if not nc.is_finalized():
        nc.finalize()
    return nc


def build_nc_v6(rows: int = ROWS, ncols: int = C, x_bufs: int = 20,
                repeat: int = 1, variant: str = "full", stt_mod: int = 3,
                e_bufs: int = 4):
    """v6: fp16 exp tiles so every DVE op runs in a 2-byte perf mode.

    Per tile: ACT exp fp32->fp16 (accum fp32 -> s), DVE InstMax top-8 on the
    fp16 e tile.  e_y: pool indirect-DMA gather of x_y (exp'd in the
    epilogue, bit-identically via the same ACT path) for most tiles, fp16
    one-hot stt accum (= e_y directly, exact) on every stt_mod-th tile.
    Epilogue identifies y == argmax by e_y == m1 (exact in fp16) and picks
    m2 in that case.  y input layout identical to v5 ([P, 2T] int32).
    """
    ntiles = rows // P
    nc = bacc.Bacc("TRN2", num_swdge_queues=4)

    x_d = nc.dram_tensor("x", [rows, ncols], mybir.dt.float32, kind="ExternalInput")
    y_d = nc.dram_tensor("y", [P, 2 * ntiles], mybir.dt.int32, kind="ExternalInput")
    out_d = nc.dram_tensor("out", [P, ntiles], mybir.dt.float32, kind="ExternalOutput")
    x_flat = x_d.rearrange("r (c one) -> (r c) one", one=1)

    fp32 = mybir.dt.float32
    fp16 = mybir.dt.float16
    i32 = mybir.dt.int32
    Alu = mybir.AluOpType

    def is_stt(t):
        return stt_mod > 0 and t % stt_mod == stt_mod - 1

    with tile.TileContext(nc) as tc:
        with (
            tc.tile_pool(name="xs", bufs=x_bufs) as xs_pool,
            tc.tile_pool(name="es", bufs=e_bufs) as es_pool,
            tc.tile_pool(name="const", bufs=1) as const_pool,
            tc.tile_pool(name="acc", bufs=1) as acc_pool,
        ):
            y2_sb = const_pool.tile([P, 2 * ntiles], i32)
            nc.sync.dma_start(out=y2_sb[:, :], in_=y_d[:, :])
            off_sb = y2_sb[:, 0:ntiles]
            y_f = const_pool.tile([P, ntiles], fp32)
            nc.vector.tensor_copy(y_f[:, :], y2_sb[:, ntiles:2 * ntiles])

            iota_i = const_pool.tile([P, ncols], i32)
            nc.gpsimd.iota(iota_i[:, :], [[1, ncols]], channel_multiplier=0)
            iota16 = const_pool.tile([P, ncols], fp16)
            nc.vector.tensor_copy(iota16[:, :], iota_i[:, :])

            s_acc = acc_pool.tile([P, ntiles], fp32)
            top8 = acc_pool.tile([P, 8 * ntiles], fp16)
            xg_g = acc_pool.tile([P, ntiles], fp32)   # gathered x_y (-1e4 at stt cols)
            ey_s = acc_pool.tile([P, ntiles], fp32)   # stt-path e_y (0 at gather cols)
            dummy16 = const_pool.tile([P, 8], fp16)

            def emit_body():
                nc.vector.memset(xg_g[:, :], -10000.0)
                nc.vector.memset(ey_s[:, :], 0.0)
                if variant in ("full", "dma_dve"):
                    for t in range(ntiles):
                        if not is_stt(t):
                            nc.gpsimd.indirect_dma_start(
                                out=xg_g[:, t:t + 1],
                                out_offset=None,
                                in_=x_flat,
                                in_offset=bass.IndirectOffsetOnAxis(
                                    ap=off_sb[:, t:t + 1], axis=0),
                            )
                for t in range(ntiles):
                    xt = xs_pool.tile([P, ncols], fp32)
                    nc.sync.dma_start(out=xt[:, :], in_=x_d[t * P:(t + 1) * P, :])

                    if variant in ("full", "dma_act"):
                        et = es_pool.tile([P, ncols], fp16)
                        nc.scalar.activation(
                            out=et[:, :],
                            in_=xt[:, :],
                            func=mybir.ActivationFunctionType.Exp,
                            accum_out=s_acc[:, t:t + 1],
                        )
                    if variant == "full":
                        nc.vector.max(out=top8[:, 8 * t:8 * t + 8], in_=et[:, :])
                        if is_stt(t):
                            nc.vector.scalar_tensor_tensor(
                                out=dummy16[:, 0:1].broadcast_to((P, ncols)),
                                in0=iota16[:, :],
                                scalar=y_f[:, t:t + 1],
                                in1=et[:, :],
                                op0=Alu.is_equal,
                                op1=Alu.mult,
                                accum_out=ey_s[:, t:t + 1],
                            )

                # Batched epilogue over [P, ntiles]:
                ey_g16 = acc_pool.tile([P, ntiles], fp16)
                nc.scalar.activation(ey_g16[:, :], xg_g[:, :],
                                     func=mybir.ActivationFunctionType.Exp)
                ey_g = acc_pool.tile([P, ntiles], fp32)
                nc.vector.tensor_copy(ey_g[:, :], ey_g16[:, :])
                ey = acc_pool.tile([P, ntiles], fp32)
                nc.vector.tensor_tensor(ey[:, :], ey_g[:, :], ey_s[:, :], Alu.add)

                t8v = top8[:, :].rearrange("p (t e) -> p t e", e=8)
                m1f = acc_pool.tile([P, ntiles], fp32)
                nc.vector.tensor_copy(m1f[:, :], t8v[:, :, 0])
                m2f = acc_pool.tile([P, ntiles], fp32)
                nc.vector.tensor_copy(m2f[:, :], t8v[:, :, 1])

                eq = acc_pool.tile([P, ntiles], fp32)
                nc.vector.tensor_tensor(eq[:, :], ey[:, :], m1f[:, :], Alu.is_equal)
                d12 = acc_pool.tile([P, ntiles], fp32)
                nc.vector.tensor_tensor(d12[:, :], m1f[:, :], m2f[:, :], Alu.subtract)
                eqd = acc_pool.tile([P, ntiles], fp32)
                nc.vector.tensor_tensor(eqd[:, :], eq[:, :], d12[:, :], Alu.mult)
                w = acc_pool.tile([P, ntiles], fp32)
                nc.vector.tensor_tensor(w[:, :], m1f[:, :], eqd[:, :], Alu.subtract)

                num = acc_pool.tile([P, ntiles], fp32)
                nc.vector.tensor_tensor(num[:, :], w[:, :], ey[:, :], Alu.subtract)
                rcp = acc_pool.tile([P, ntiles], fp32)
                nc.vector.reciprocal(rcp[:, :], s_acc[:, :])
                prod = acc_pool.tile([P, ntiles], fp32)
                nc.vector.tensor_tensor(prod[:, :], num[:, :], rcp[:, :], Alu.mult)
                outb = acc_pool.tile([P, ntiles], fp32)
                nc.vector.tensor_scalar(
                    out=outb[:, :], in0=prod[:, :], scalar1=1.0, scalar2=None,
                    op0=Alu.add,
                )
                nc.sync.dma_start(out=out_d[:, :], in_=outb[:, :])

            if variant in ("dma", "dma_act", "dma_dve"):
                nc.vector.memset(s_acc[:, :], 1.0)
                nc.vector.memset(top8[:, :], 1.0)
            if repeat > 1:
                with tc.For_i(0, repeat, 1):
                    emit_body()
            else:
                emit_body()
    if not nc.is_finalized():
        nc.finalize()
    return nc


def build_nc_v7(rows: int = ROWS, ncols: int = C, x_bufs: int = 6,
                repeat: int = 1, variant: str = "full", stt_mod: int = 0,
                e_bufs: int = 3, rpp: int = 4, dma2q: bool = False):
    """v7: v6 compute with big DMA transfers (rpp rows per partition).

    One dma_start moves rpp*512KB: partition p holds rows rpp*P*bt + rpp*p
    .. +rpp-1 contiguously (8/16KB per partition line).  Compute runs per
    1000-col slice exactly as v6; out col c = rpp*bt + h covers rows
    rpp*P*bt + rpp*p + h.  Host marshalling must use y_dtype="v7-<rpp>".
    """
    ntiles = rows // P
    nbig = ntiles // rpp
    nc = bacc.Bacc("TRN2", num_swdge_queues=4)

    x_d = nc.dram_tensor("x", [rows, ncols], mybir.dt.float32, kind="ExternalInput")
    y_d = nc.dram_tensor("y", [P, 2 * ntiles], mybir.dt.int32, kind="ExternalInput")
    out_d = nc.dram_tensor("out", [P, ntiles], mybir.dt.float32, kind="ExternalOutput")
    x_flat = x_d.rearrange("r (c one) -> (r c) one", one=1)
    xb = x_d.rearrange("(n p k) c -> n p (k c)", p=P, k=rpp)

    fp32 = mybir.dt.float32
    fp16 = mybir.dt.float16
    i32 = mybir.dt.int32
    Alu = mybir.AluOpType

    def is_stt(t):
        return stt_mod > 0 and t % stt_mod == stt_mod - 1

    with tile.TileContext(nc) as tc:
        with (
            tc.tile_pool(name="xs", bufs=x_bufs) as xs_pool,
            tc.tile_pool(name="es", bufs=e_bufs) as es_pool,
            tc.tile_pool(name="const", bufs=1) as const_pool,
            tc.tile_pool(name="acc", bufs=1) as acc_pool,
        ):
            y2_sb = const_pool.tile([P, 2 * ntiles], i32)
            nc.sync.dma_start(out=y2_sb[:, :], in_=y_d[:, :])
            off_sb = y2_sb[:, 0:ntiles]
            y_f = const_pool.tile([P, ntiles], fp32)
            nc.vector.tensor_copy(y_f[:, :], y2_sb[:, ntiles:2 * ntiles])

            iota_i = const_pool.tile([P, ncols], i32)
            nc.gpsimd.iota(iota_i[:, :], [[1, ncols]], channel_multiplier=0)
            iota16 = const_pool.tile([P, ncols], fp16)
            nc.vector.tensor_copy(iota16[:, :], iota_i[:, :])

            s_acc = acc_pool.tile([P, ntiles], fp32)
            top8 = acc_pool.tile([P, 8 * ntiles], fp16)
            xg_g = acc_pool.tile([P, ntiles], fp32)
            ey_s = acc_pool.tile([P, ntiles], fp32)
            dummy16 = const_pool.tile([P, 8], fp16)

            def emit_body():
                nc.vector.memset(xg_g[:, :], -10000.0)
                nc.vector.memset(ey_s[:, :], 0.0)
                if variant in ("full", "dma_dve"):
                    for t in range(ntiles):
                        if not is_stt(t):
                            nc.gpsimd.indirect_dma_start(
                                out=xg_g[:, t:t + 1],
                                out_offset=None,
                                in_=x_flat,
                                in_offset=bass.IndirectOffsetOnAxis(
                                    ap=off_sb[:, t:t + 1], axis=0),
                            )
                for bt in range(nbig):
                    xt = xs_pool.tile([P, rpp * ncols], fp32)
                    eng = nc.scalar if (dma2q and bt % 2) else nc.sync
                    eng.dma_start(out=xt[:, :], in_=xb[bt])

                    if variant in ("full", "dma_act"):
                        et = es_pool.tile([P, rpp * ncols], fp16)
                        for h in range(rpp):
                            t = rpp * bt + h
                            sl = slice(h * ncols, (h + 1) * ncols)
                            nc.scalar.activation(
                                out=et[:, sl],
                                in_=xt[:, sl],
                                func=mybir.ActivationFunctionType.Exp,
                                accum_out=s_acc[:, t:t + 1],
                            )
                    if variant == "full":
                        for h in range(rpp):
                            t = rpp * bt + h
                            sl = slice(h * ncols, (h + 1) * ncols)
                            nc.vector.max(out=top8[:, 8 * t:8 * t + 8],
                                          in_=et[:, sl])
                            if is_stt(t):
                                nc.vector.scalar_tensor_tensor(
                                    out=dummy16[:, 0:1].broadcast_to((P, ncols)),
                                    in0=iota16[:, :],
                                    scalar=y_f[:, t:t + 1],
                                    in1=et[:, sl],
                                    op0=Alu.is_equal,
                                    op1=Alu.mult,
                                    accum_out=ey_s[:, t:t + 1],
                                )

                # Batched epilogue over [P, ntiles]:
                ey_g16 = acc_pool.tile([P, ntiles], fp16)
                nc.scalar.activation(ey_g16[:, :], xg_g[:, :],
                                     func=mybir.ActivationFunctionType.Exp)
                ey_g = acc_pool.tile([P, ntiles], fp32)
                nc.vector.tensor_copy(ey_g[:, :], ey_g16[:, :])
                ey = acc_pool.tile([P, ntiles], fp32)
                nc.vector.tensor_tensor(ey[:, :], ey_g[:, :], ey_s[:, :], Alu.add)

                t8v = top8[:, :].rearrange("p (t e) -> p t e", e=8)
                m1f = acc_pool.tile([P, ntiles], fp32)
                nc.vector.tensor_copy(m1f[:, :], t8v[:, :, 0])
                m2f = acc_pool.tile([P, ntiles], fp32)
                nc.vector.tensor_copy(m2f[:, :], t8v[:, :, 1])

                eq = acc_pool.tile([P, ntiles], fp32)
                nc.vector.tensor_tensor(eq[:, :], ey[:, :], m1f[:, :], Alu.is_equal)
                d12 = acc_pool.tile([P, ntiles], fp32)
                nc.vector.tensor_tensor(d12[:, :], m1f[:, :], m2f[:, :], Alu.subtract)
                eqd = acc_pool.tile([P, ntiles], fp32)
                nc.vector.tensor_tensor(eqd[:, :], eq[:, :], d12[:, :], Alu.mult)
                w = acc_pool.tile([P, ntiles], fp32)
                nc.vector.tensor_tensor(w[:, :], m1f[:, :], eqd[:, :], Alu.subtract)

                num = acc_pool.tile([P, ntiles], fp32)
                nc.vector.tensor_tensor(num[:, :], w[:, :], ey[:, :], Alu.subtract)
                rcp = acc_pool.tile([P, ntiles], fp32)
                nc.vector.reciprocal(rcp[:, :], s_acc[:, :])
                prod = acc_pool.tile([P, ntiles], fp32)
                nc.vector.tensor_tensor(prod[:, :], num[:, :], rcp[:, :], Alu.mult)
                outb = acc_pool.tile([P, ntiles], fp32)
                nc.vector.tensor_scalar(
                    out=outb[:, :], in0=prod[:, :], scalar1=1.0, scalar2=None,
                    op0=Alu.add,
                )
                nc.sync.dma_start(out=out_d[:, :], in_=outb[:, :])

            if variant in ("dma", "dma_act", "dma_dve"):
                nc.vector.memset(s_acc[:, :], 1.0)
                nc.vector.memset(top8[:, :], 1.0)
            if repeat > 1:
                with tc.For_i(0, repeat, 1):
                    emit_body()
            else:
                emit_body()
    if not nc.is_finalized():
        nc.finalize()
    return nc


def build_nc_v8(rows: int = ROWS, ncols: int = C, x_bufs: int = 6,
                repeat: int = 1, variant: str = "full", rpp: int = 4,
                e_bufs: int = 3, gsz: int = 8):
    """v8: big DMA + fp16 exp/InstMax + on-chip e_y via POOL indirect_copy.

    Per group of gsz tiles (= gsz/rpp big DMA tiles): ACT exp fp32->fp16
    into one contiguous e_all tile (accum fp32 -> s per tile), DVE InstMax
    top-8 per tile slice, then ONE gpsimd.indirect_copy gathers, for each
    16-partition block, all 16 members' y-columns (indices shared per
    block, wrapped layout: out[p, L] = e_all[p, idx[16*(p//16)+L%16, L//16]]).
    The diagonal (l == p%16) is the row's own e_y; a constant host-provided
    mask + tensor_reduce extracts it exactly.  No DRAM gathers, no stt.

    Inputs: x [rows, ncols] fp32; y [P, T] uint16 with
    y[p, t] = (t % gsz)*ncols + label(row(p, t)); m [P, gsz*16] fp16 mask
    m[p, c] = (c % 16 == p % 16).  Row mapping as v7 (rpp rows/partition).
    """
    ntiles = rows // P
    nbig = ntiles // rpp
    ngroups = ntiles // gsz
    big_per_g = gsz // rpp
    nc = bacc.Bacc("TRN2")

    x_d = nc.dram_tensor("x", [rows, ncols], mybir.dt.float32, kind="ExternalInput")
    y_d = nc.dram_tensor("y", [P, ntiles], mybir.dt.uint16, kind="ExternalInput")
    m_d = nc.dram_tensor("m", [P, gsz * 16], mybir.dt.float16,
                         kind="ExternalInput")
    out_d = nc.dram_tensor("out", [P, ntiles], mybir.dt.float32, kind="ExternalOutput")
    xb = x_d.rearrange("(n p k) c -> n p (k c)", p=P, k=rpp)

    fp32 = mybir.dt.float32
    fp16 = mybir.dt.float16
    Alu = mybir.AluOpType

    with tile.TileContext(nc) as tc:
        with (
            tc.tile_pool(name="xs", bufs=x_bufs) as xs_pool,
            tc.tile_pool(name="es", bufs=e_bufs) as es_pool,
            tc.tile_pool(name="od", bufs=4) as od_pool,
            tc.tile_pool(name="const", bufs=1) as const_pool,
            tc.tile_pool(name="acc", bufs=1) as acc_pool,
        ):
            yi_sb = const_pool.tile([P, ntiles], mybir.dt.uint16)
            nc.sync.dma_start(out=yi_sb[:, :], in_=y_d[:, :])
            mk_sb = const_pool.tile([P, gsz * 16], fp16)
            nc.sync.dma_start(out=mk_sb[:, :], in_=m_d[:, :])

            s_acc = acc_pool.tile([P, ntiles], fp32)
            top8 = acc_pool.tile([P, 8 * ntiles], fp16)
            ey = acc_pool.tile([P, ntiles], fp32)

            def emit_body():
                for g in range(ngroups):
                    e_all = es_pool.tile([P, gsz * ncols], fp16)
                    for b in range(big_per_g):
                        bt = big_per_g * g + b
                        xt = xs_pool.tile([P, rpp * ncols], fp32)
                        nc.sync.dma_start(out=xt[:, :], in_=xb[bt])
                        if variant == "dma":
                            continue
                        for h in range(rpp):
                            j = b * rpp + h
                            t = gsz * g + j
                            nc.scalar.activation(
                                out=e_all[:, j * ncols:(j + 1) * ncols],
                                in_=xt[:, h * ncols:(h + 1) * ncols],
                                func=mybir.ActivationFunctionType.Exp,
                                accum_out=s_acc[:, t:t + 1],
                            )
                            if variant in ("full", "dma_act_max"):
                                nc.vector.max(
                                    out=top8[:, 8 * t:8 * t + 8],
                                    in_=e_all[:, j * ncols:(j + 1) * ncols])
                    if variant == "full":
                        oid = od_pool.tile([P, gsz * 16], fp16)
                        nc.gpsimd.indirect_copy(
                            out=oid[:, :], data=e_all[:, :],
                            idxs=yi_sb[:, gsz * g:gsz * (g + 1)],
                            i_know_ap_gather_is_preferred=True)
                        tmp = od_pool.tile([P, gsz * 16], fp16)
                        nc.vector.tensor_tensor(
                            tmp[:, :], oid[:, :], mk_sb[:, :], Alu.mult)
                        nc.vector.tensor_reduce(
                            out=ey[:, gsz * g:gsz * (g + 1)],
                            in_=tmp[:, :].rearrange("p (j l) -> p j l", l=16),
                            axis=mybir.AxisListType.X,
                            op=Alu.add,
                        )

                # Batched epilogue over [P, ntiles]:
                t8v = top8[:, :].rearrange("p (t e) -> p t e", e=8)
                m1f = acc_pool.tile([P, ntiles], fp32)
                nc.vector.tensor_copy(m1f[:, :], t8v[:, :, 0])
                m2f = acc_pool.tile([P, ntiles], fp32)
                nc.vector.tensor_copy(m2f[:, :], t8v[:, :, 1])

                eq = acc_pool.tile([P, ntiles], fp32)
                nc.vector.tensor_tensor(eq[:, :], ey[:, :], m1f[:, :], Alu.is_equal)
                d12 = acc_pool.tile([P, ntiles], fp32)
                nc.vector.tensor_tensor(d12[:, :], m1f[:, :], m2f[:, :], Alu.subtract)
                eqd = acc_pool.tile([P, ntiles], fp32)
                nc.vector.tensor_tensor(eqd[:, :], eq[:, :], d12[:, :], Alu.mult)
                w = acc_pool.tile([P, ntiles], fp32)
                nc.vector.tensor_tensor(w[:, :], m1f[:, :], eqd[:, :], Alu.subtract)

                num = acc_pool.tile([P, ntiles], fp32)
                nc.vector.tensor_tensor(num[:, :], w[:, :], ey[:, :], Alu.subtract)
                rcp = acc_pool.tile([P, ntiles], fp32)
                nc.vector.reciprocal(rcp[:, :], s_acc[:, :])
                prod = acc_pool.tile([P, ntiles], fp32)
                nc.vector.tensor_tensor(prod[:, :], num[:, :], rcp[:, :], Alu.mult)
                outb = acc_pool.tile([P, ntiles], fp32)
                nc.vector.tensor_scalar(
                    out=outb[:, :], in0=prod[:, :], scalar1=1.0, scalar2=None,
                    op0=Alu.add,
                )
                nc.sync.dma_start(out=out_d[:, :], in_=outb[:, :])

            if variant in ("dma", "dma_act", "dma_act_max"):
                nc.vector.memset(s_acc[:, :], 1.0)
                nc.vector.memset(top8[:, :], 1.0)
                nc.vector.memset(ey[:, :], 1.0)
            if repeat > 1:
                with tc.For_i(0, repeat, 1):
                    emit_body()
            else:
                emit_body()
    if not nc.is_finalized():
        nc.finalize()
    return nc


def make_in_maps(state_output: np.ndarray, y: np.ndarray, y_dtype=np.int32,
                 rows: int = ROWS, ncols: int = C):
    """Shard the full inputs across cores (batch split + y marshalling).

    For the v3 kernel (y_dtype=int32) the y input carries flat element
    offsets local_row*ncols + y, the gather table for the indirect DMA.
    """
    x_full = np.ascontiguousarray(np.asarray(state_output, dtype=np.float32))
    y_full = np.asarray(y)
    ncores = y_full.shape[0] // rows
    in_maps = []
    for i in range(ncores):
        lo, hi = i * rows, (i + 1) * rows
        y_shard = y_full[lo:hi]
        if isinstance(y_dtype, str) and y_dtype.startswith("v8-"):
            rpp = int(y_dtype.split("-")[1])
            gsz = 8
            ntiles = rows // P
            r_mat = row_map_v7(rows, rpp)                   # [P, T] row index
            yv = y_shard.astype(np.int64)[r_mat]            # label per (p, t)
            ji = (np.arange(ntiles) % gsz)[None, :]
            yi = (ji * ncols + yv).astype(np.uint16)        # [P, T]
            mk = (np.arange(gsz * 16)[None, :] % 16
                  == np.arange(P)[:, None] % 16).astype(np.float16)
            in_maps.append({"x": x_full[lo:hi], "y": yi, "m": mk})
            continue
        if isinstance(y_dtype, str) and y_dtype.startswith("v7-"):
            rpp = int(y_dtype.split("-")[1])
            ntiles = rows // P
            r_mat = row_map_v7(rows, rpp)                   # [P, T] row index
            offs = (r_mat.astype(np.int64) * ncols
                    + y_shard.astype(np.int64)[r_mat]).astype(np.int32)
            vals = y_shard.astype(np.int32)[r_mat]
            y_t = np.ascontiguousarray(
                np.concatenate([offs, vals], axis=1))       # [P, 2T]
        elif y_dtype == "v5":
            offs = (np.arange(rows, dtype=np.int64) * ncols
                    + y_shard.astype(np.int64)).astype(np.int32)
            o_t = offs.reshape(rows // P, P).T              # [P, T]
            v_t = y_shard.astype(np.int32).reshape(rows // P, P).T
            y_t = np.ascontiguousarray(np.concatenate([o_t, v_t], axis=1))
        elif y_dtype == np.int32:
            vals = (np.arange(rows, dtype=np.int64) * ncols
                    + y_shard.astype(np.int64)).astype(np.int32)
            y_t = np.ascontiguousarray(vals.reshape(rows // P, P).T)  # [P, T]
        else:
            vals = y_shard.astype(y_dtype)
            y_t = np.ascontiguousarray(vals.reshape(rows // P, P).T)  # [P, T]
        in_maps.append({"x": x_full[lo:hi], "y": y_t})
    return in_maps


KERNEL_VERSION = "v8-4"   # "v5" / "v3" / "v6" / "v7-<rpp>" fallbacks


def row_map_v7(rows: int, rpp: int) -> np.ndarray:
    """[P, T] matrix: local row index held at (partition p, out col c)."""
    ntiles = rows // P
    bt = np.arange(ntiles) // rpp
    h = np.arange(ntiles) % rpp
    return (rpp * P * bt)[None, :] + rpp * np.arange(P)[:, None] + h[None, :]


def y_dtype_for(version: str):
    if version.startswith(("v7-", "v8-")):
        return version
    return "v5" if version in ("v5", "v6") else np.int32


def build_for(version: str, **kw):
    if version.startswith("v7-"):
        return build_nc_v7(rpp=int(version.split("-")[1]), **kw)
    if version.startswith("v8-"):
        return build_nc_v8(rpp=int(version.split("-")[1]), **kw)
    return globals()[f"build_nc_{version}"](**kw)


def unshard_out(o: np.ndarray, version: str) -> np.ndarray:
    """Per-core [P, T] device output -> flat [ROWS] local losses."""
    if version.startswith(("v7-", "v8-")):
        rpp = int(version.split("-")[1])
        ntiles = o.shape[1]
        o3 = o.reshape(P, ntiles // rpp, rpp)
        return o3.transpose(1, 0, 2).reshape(-1)
    return o.T.reshape(-1)


def kernel(state_output: np.ndarray, y: np.ndarray) -> np.ndarray:
    if "nc" not in _cache:
        _cache["nc"] = build_for(KERNEL_VERSION)
    nc = _cache["nc"]
    in_maps = make_in_maps(state_output, y, y_dtype=y_dtype_for(KERNEL_VERSION))
    res = run_bass_kernel_spmd(nc, in_maps, core_ids=list(range(NCORES)))
    outs = []
    for i in range(NCORES):
        o = np.asarray(res.results[i]["out"])  # [P, T]
        outs.append(unshard_out(o, KERNEL_VERSION))
    return np.concatenate(outs).astype(np.float32)

